# revision 14
# baseline (speedup 1.0000x reference)
"""Trainium2 Bass kernel for the CCA module (attention + 1x1 convs + BN/ReLU).

Contract: kernel(**inputs) takes the FULL fp32 inputs (shapes hardcoded below),
shards the batch over 8 NeuronCores (2 samples each), runs a Bass/Tile kernel
via run_bass_kernel_spmd, and returns the FULL (16, 512, 64, 64) fp32 output.

Host-side preprocessing (numpy):
  - BN (eval mode) folded into the 1x1 conv weights/biases.
  - x shipped twice: fp8-e4m3 (x8, nt-sliced) as the projT stationary -- fp8
    LDWEIGHTS runs FWL at 4 cols/cycle (2x bf16) and halves the head DMA --
    and bf16 in pair-major layout [128, NCH, 2, 4, 512] as the c2 moving
    operand (c2 needs bf16 precision; fp8 there costs 2.6e-2 rel err).
  - attT (pixel-partitioned) and att2 (pixel halves stacked into 128
    partitions) shipped fp8-e4m3: att only feeds energy (softmax-robust) and
    out2 (~3.5%-magnitude contribution to y). Measured end-to-end rel err of
    all fp8 paths together: ~7e-3 (gate 2e-2).
  - y returned bf16, upcast on host.

Device-side per sample s (C=512, C8=64, HW=4096 pixels):
  projT[n,k] = sum_c x[c,n] * key_w[k,c]      (x8 tiles stationary)
  energy[k,q] = sum_n projT[n,k] * attT[n,q]  (accumulated per x-quarter)
  attn = softmax_q(energy)                    (max/exp/sum on ACT+DVE)
  w1aT_q = attn^T @ [w1T|w1T], duplicated into both partition halves
  out2_full = relu(W1a @ att + b1) as [128, HW/2]: both pixel halves at once
  y[o,n] = relu(sum_c W2b[o,c] x[c,n] + sum_k W2a[o,k] out2[k,n] + b2[o])
           (x-part K=128 chains pairwise over (j, j+4); out2-part K=64
            matmuls run pairwise concurrent in opposite PE row halves)

Schedule: the head (sample-0 attention) is DMA-paced. DMA queues cap at
~160 GB/s each (~400 aggregate), so every large tensor is split across
queues and issued in need-order. Warmup + filler matmuls keep the HAM clock
gate open through the head's DMA gaps (one idle MID-window re-throttles the
PE to 1.2 GHz). Sample-1 attention interleaves into sample-0's c2 stream.
The last two output tiles store at quarter/pair granularity to cut the tail.
"""

from contextlib import ExitStack

import numpy as np

import concourse.bacc as bacc
import concourse.tile as tile
from concourse import mybir
from concourse.bass_utils import run_bass_kernel_spmd

N_CORES = 8
B, C, H, W = 16, 512, 64, 64
C8 = C // 8          # 64
HW = H * W           # 4096
S = B // N_CORES     # samples per core = 2
NCH = C // 128       # channel chunks = 4
NT = HW // 128       # 128-wide pixel tiles = 32
EPS = 1e-5
NWARM = 26           # 128-col warmup matmuls: pre-warm the HAM clock gate
                     # while the first x8 piece streams in
NFILL = 4            # 512-col keep-alive matmuls per head DMA gap

BF16 = mybir.dt.bfloat16
F32 = mybir.dt.float32
FP8 = mybir.dt.float8e4
NP_BF16 = mybir.dt.np(BF16)
NP_FP8 = mybir.dt.np(FP8)

_BUILT = None
PHASE_MARKS = []  # (label, n_insts_at_mark) for trace attribution


def _mark(nc, label):
    PHASE_MARKS.append((label, len(nc.inst_map)))


class _Ctx:
    """Bag of state shared by the emission helpers."""
    pass


def _xtile_bf(x_sb, ci, nt):
    """[128, 128] projT stationary tile nt (0..31) from the pair-major bf16
    x buffer (memory layout is pixel-linear, so the slice is rectangular)."""
    return x_sb[:, ci, nt // 16, (nt % 16) // 4,
                (nt % 4) * 128:(nt % 4 + 1) * 128]


def _emit_projT_quarter(k, s, q, xtile):
    """projT for pixel quarter q (8 n-tiles); returns the evacuated bf16 tile.

    xtile(ci, nt) -> [128, 128] stationary AP."""
    nc = k.nc
    _mark(nc, f"projT_{s}_{q}")
    pA = k.psA.tile([128, 8, C8], F32, name=f"pA_{s}_{q}", tag="pa")
    for i in range(8):
        nt = q * 8 + i
        for ci in range(NCH):
            nc.tensor.matmul(
                pA[:, i, :],
                lhsT=xtile(ci, nt),
                rhs=k.sb["kwT"][:, ci, :],
                start=(ci == 0), stop=(ci == NCH - 1))
    pj = k.projTpool.tile([128, 8, C8], BF16, name=f"pj_{s}_{q}", tag="pj",
                          bufs=8)
    # evacuate in halves: the first 4 n-tiles' energy matmuls can start
    # while the second half is still being evacuated
    nc.vector.tensor_add(pj[:, 0:4, :], pA[:, 0:4, :], k.sb["kb_bc"][:, 0:4, :])
    nc.vector.tensor_add(pj[:, 4:8, :], pA[:, 4:8, :], k.sb["kb_bc"][:, 4:8, :])
    return pj


def _emit_energy_quarter(k, s, q, pj, attT, ps_e):
    nc = k.nc
    _mark(nc, f"energy_{s}_{q}")
    for i in range(8):
        nt = q * 8 + i
        nc.tensor.matmul(ps_e, lhsT=pj[:, i, :], rhs=attT[:, nt, :],
                         start=(q == 0 and i == 0), stop=(q == 3 and i == 7))


def _emit_c2x_pair(k, ps_l, ps_h, ot, jj, x_sb, start=True, stop=False):
    """x-only chains for blocks jj (lo half) and jj+4 (hi half), sharing
    each LDWEIGHTS."""
    nc = k.nc
    for ci in range(NCH):
        w = k.sb["w2bT"][:, ot, ci, :]
        nc.tensor.matmul(ps_l, lhsT=w, rhs=x_sb[:, ci, 0, jj, :],
                         start=(start and ci == 0),
                         stop=(stop and ci == NCH - 1))
        nc.tensor.matmul(ps_h, lhsT=w, rhs=x_sb[:, ci, 1, jj, :],
                         start=(start and ci == 0),
                         stop=(stop and ci == NCH - 1))


def _emit_c2o_mm(k, ps, ot, j, out2_full, hi, start=False, stop=True):
    """out2-part matmul for linear block j; hi selects the upper PE row half."""
    nc = k.nc
    if hi:
        nc.tensor.matmul(
            ps, lhsT=k.sb["wa_blob"][64:128, ot * 128:(ot + 1) * 128],
            rhs=out2_full[64:128, (j - 4) * 512:(j - 3) * 512],
            start=start, stop=stop)
    else:
        nc.tensor.matmul(
            ps, lhsT=k.sb["wa_blob"][0:64, ot * 128:(ot + 1) * 128],
            rhs=out2_full[0:64, j * 512:(j + 1) * 512],
            start=start, stop=stop)


def _emit_y_evac(k, s, ps, ot, j, y_sb, use_act):
    """PSUM -> y_sb bf16 with +b2 and relu; alternate ACT/DVE."""
    nc = k.nc
    dst = y_sb[:, j // 4, j % 4, :]
    b2col = k.sb["bias_blob"][:, 1 + ot:2 + ot]
    if use_act:
        nc.scalar.activation(dst, ps, mybir.ActivationFunctionType.Relu,
                             bias=b2col, scale=1.0)
    else:
        nc.vector.tensor_scalar(out=dst, in0=ps,
                                scalar1=b2col, scalar2=0.0,
                                op0=mybir.AluOpType.add,
                                op1=mybir.AluOpType.max)


def _emit_filler(k, n, cols):
    """HAM keep-alive: dummy matmuls so the PE clock gate stays open while
    the engine is data-starved during the load phase."""
    nc = k.nc
    ps = k.psA.tile([128, 512], F32, name="filler_ps", tag="pa")
    for _ in range(n):
        nc.tensor.matmul(ps[:, 0:cols], lhsT=k.sb["warm"][:, 0:128],
                         rhs=k.sb["warm"][:, 0:cols], start=True, stop=True)


def _emit_softmax(k, s, ps_e):
    """softmax + W1a for sample s (after energy accumulation); returns the
    duplicated w1aT stationary for the out2 matmuls."""
    nc = k.nc
    _mark(nc, f"softmax_{s}")
    negmax = k.small.tile([C8, 1], F32, name=f"negmax_{s}")
    nc.vector.tensor_reduce(negmax, ps_e, axis=mybir.AxisListType.X,
                            op=mybir.AluOpType.max, negate=True)
    attn_exp = k.small.tile([C8, C8], F32, name=f"attn_exp_{s}")
    sumexp = k.small.tile([C8, 1], F32, name=f"sumexp_{s}")
    nc.scalar.activation(attn_exp, ps_e, mybir.ActivationFunctionType.Exp,
                         bias=negmax, scale=1.0, accum_out=sumexp)
    rec = k.small.tile([C8, 1], F32, name=f"rec_{s}")
    nc.vector.reciprocal(rec, sumexp)
    attn_dup = k.small.tile([C8, 128], BF16, name=f"attn_dup_{s}")
    nc.vector.tensor_scalar_mul(attn_dup[:, 0:C8], attn_exp, rec)
    nc.vector.tensor_scalar_mul(attn_dup[:, C8:128], attn_exp, rec)

    # w1aT duplicated along both output cols and partition halves in one MM
    ps_w2 = k.psB.tile([128, 128], F32, name=f"ps_w2_{s}", tag="sm")
    nc.tensor.matmul(ps_w2, lhsT=attn_dup, rhs=k.sb["wa_blob"][0:64, 512:640],
                     start=True, stop=True)
    w1aT_q = k.small.tile([128, 128], BF16, name=f"w1aT_q_{s}")
    nc.scalar.copy(w1aT_q, ps_w2)
    return w1aT_q


def _emit_out2_jb(k, s, w1aT_q, att2, out2_full, jb):
    """out2 = relu(W1a @ att + b1) for 512-col block jb, both pixel halves
    row-concurrent."""
    nc = k.nc
    sl = slice(jb * 512, (jb + 1) * 512)
    ps_lo = k.psB.tile([128, 512], F32, name=f"ps_o_lo_{s}_{jb}", tag="sm")
    ps_hi = k.psB.tile([128, 512], F32, name=f"ps_o_hi_{s}_{jb}", tag="sm")
    nc.tensor.matmul(ps_lo, lhsT=w1aT_q[0:64, :], rhs=att2[0:64, sl],
                     start=True, stop=True)
    nc.tensor.matmul(ps_hi, lhsT=w1aT_q[64:128, :], rhs=att2[64:128, sl],
                     start=True, stop=True)
    nc.scalar.activation(out2_full[0:64, sl], ps_lo[0:64, :],
                         mybir.ActivationFunctionType.Relu,
                         bias=k.sb["bias_blob"][0:64, 0:1], scale=1.0)
    nc.vector.tensor_scalar(out=out2_full[64:128, sl], in0=ps_hi[64:128, :],
                            scalar1=k.sb["bias_blob"][64:128, 0:1],
                            scalar2=0.0,
                            op0=mybir.AluOpType.add,
                            op1=mybir.AluOpType.max)


def _get_y(k, s, ot):
    key = (s, ot)
    if key not in k.ytiles:
        k.ytiles[key] = k.ypool.tile([128, 2, 4, 512], BF16,
                                     name=f"y_sb_{s}_{ot}", tag="y", bufs=5)
    return k.ytiles[key]


def _maybe_store(k, s, ot, y_sb, quarters=False, pair_stores=False,
                 done_jj=None, split_last=False):
    nc = k.nc
    done = k.jdone[(s, ot)]
    if pair_stores:
        # store the (j, j+4) pair's two 512-col spans as one strided DMA as
        # soon as both are evacuated, so the final drain starts immediately.
        # The very last pair stores its halves separately: the lo half's DMA
        # overlaps the hi half's evacuation.
        if done_jj is None or done_jj in k.stored.setdefault((s, ot), set()):
            return
        k.stored[(s, ot)].add(done_jj)
        if split_last:
            nc.sync.dma_start(out=k.dram["y"][s, ot, :, 0, done_jj, :],
                              in_=y_sb[:, 0, done_jj, :])
            nc.sync.dma_start(out=k.dram["y"][s, ot, :, 1, done_jj, :],
                              in_=y_sb[:, 1, done_jj, :])
        else:
            nc.sync.dma_start(out=k.dram["y"][s, ot, :, :, done_jj, :],
                              in_=y_sb[:, :, done_jj, :])
    elif quarters:
        # quarter-granularity: store each 1024-col span as soon as both of
        # its j-blocks are evacuated
        for qt in range(4):
            lo, hi = 2 * qt, 2 * qt + 1
            ready = (lo in done) and (hi in done)
            if ready and qt not in k.stored.setdefault((s, ot), set()):
                k.stored[(s, ot)].add(qt)
                h, u = qt // 2, qt % 2
                nc.sync.dma_start(
                    out=k.dram["y"][s, ot, :, h, 2 * u:2 * u + 2, :],
                    in_=y_sb[:, h, 2 * u:2 * u + 2, :])
    elif len(done) == 8 and (s, ot) not in k.stored:
        # one store per output tile: DMA issue costs ~0.7us of Sync-queue
        # time apiece, so keep the count low mid-kernel
        k.stored[(s, ot)] = True
        nc.sync.dma_start(out=k.dram["y"][s, ot], in_=y_sb[:, :, :, :])


def _emit_c2_ot(k, s, ot, x_sb, out2_full, skip_jj=(), wide_psum=False,
                quarters=False, pair_stores=False, last=False):
    """Full c2 for output tile ot: x chains + row-paired out2 MMs + evacs.

    wide_psum: alternate pairs between psC and psA/psB (only safe once
    projT/energy/out2 are all done with those pools) -- doubles the pair
    pipeline depth to 8 banks."""
    nc = k.nc
    _mark(nc, f"c2_{s}_{ot}")
    y_sb = _get_y(k, s, ot)
    done = k.jdone.setdefault((s, ot), [])
    for pi, jj in enumerate(jj for jj in range(4) if jj not in skip_jj):
        jl, jh = jj, jj + 4
        if wide_psum and pi % 2 == 1:
            ps_l = k.psA.tile([128, 512], F32, name=f"ps_y_{s}_{ot}_{jl}",
                              tag="pa")
            ps_h = k.psB.tile([128, 512], F32, name=f"ps_y_{s}_{ot}_{jh}",
                              tag="sm")
        else:
            ps_l = k.psC.tile([128, 512], F32, name=f"ps_y_{s}_{ot}_{jl}",
                              tag="c2")
            ps_h = k.psC.tile([128, 512], F32, name=f"ps_y_{s}_{ot}_{jh}",
                              tag="c2")
        # out2-part first: the lo/hi pair runs row-concurrent in one MM slot
        # at pair start, and the evacs then follow the last x-MM directly
        _emit_c2o_mm(k, ps_l, ot, jl, out2_full, hi=False, start=True,
                     stop=False)
        _emit_c2o_mm(k, ps_h, ot, jh, out2_full, hi=True, start=True,
                     stop=False)
        _emit_c2x_pair(k, ps_l, ps_h, ot, jj, x_sb, start=False, stop=True)
        _emit_y_evac(k, s, ps_l, ot, jl, y_sb, use_act=True)
        _emit_y_evac(k, s, ps_h, ot, jh, y_sb, use_act=False)
        done += [jl, jh]
        _maybe_store(k, s, ot, y_sb, quarters, pair_stores, done_jj=jj,
                     split_last=(last and jj == 3))


def _build():
    """Build and finalize the per-core Bass program (same on all 8 cores)."""
    PHASE_MARKS.clear()
    nc = bacc.Bacc("TRN2", target_bir_lowering=False, debug=False)

    k = _Ctx()
    k.nc = nc
    k.stored = {}
    k.jdone = {}
    k.ytiles = {}
    k.dram = {
        # x: pair-major pixel layout (h, jj, p) == linear pixel h*2048+jj*512+p
        "x": nc.dram_tensor("x", [S, 128, NCH, 2, 4, 512], BF16,
                            kind="ExternalInput"),
        # fp8 copy of sample-1's x, nt-sliced, for fast projT LDWEIGHTS
        # (FWL 4 cols/cyc); sample-0's projT reads the bf16 x directly
        # since the head is DMA-bound anyway
        "x8": nc.dram_tensor("x8", [128, NCH, NT, 128], FP8,
                             kind="ExternalInput"),
        "attT8": nc.dram_tensor("attT8", [S, 128, NT, C8], FP8,
                                kind="ExternalInput"),
        "att28": nc.dram_tensor("att28", [S, 128, HW // 2], FP8,
                                kind="ExternalInput"),
        "kwT": nc.dram_tensor("kwT", [128, NCH, C8], BF16,
                              kind="ExternalInput"),
        "kb_bc": nc.dram_tensor("kb_bc", [128, 8, C8], BF16,
                                kind="ExternalInput"),
        "wa_blob": nc.dram_tensor("wa_blob", [128, 640], BF16,
                                  kind="ExternalInput"),
        "bias_blob": nc.dram_tensor("bias_blob", [128, 5], F32,
                                    kind="ExternalInput"),
        # w2bT: per-ot contiguous chunks (strided DMA issues cost 1.5-3.3us
        # each on the Sync queue)
        "w2bT": nc.dram_tensor("w2bT", [128, 4, NCH, 128], BF16,
                               kind="ExternalInput"),
        "y": nc.dram_tensor("y", [S, 4, 128, 2, 4, 512], BF16,
                            kind="ExternalOutput"),
    }

    with nc.allow_low_precision("bf16/fp8 activations; fp32 accumulate"), \
         tile.TileContext(nc) as tc:
        with ExitStack() as ctx:
            k.consts = ctx.enter_context(tc.tile_pool(name="consts", bufs=1))
            k.xpool = ctx.enter_context(tc.tile_pool(name="xpool", bufs=2))
            k.x8pool = ctx.enter_context(tc.tile_pool(name="x8pool", bufs=1))
            k.att2pool = ctx.enter_context(tc.tile_pool(name="att2pool", bufs=2))
            k.attTpool = ctx.enter_context(tc.tile_pool(name="attTpool", bufs=2))
            k.projTpool = ctx.enter_context(tc.tile_pool(name="projTpool",
                                                         bufs=2))
            k.out2pool = ctx.enter_context(tc.tile_pool(name="out2pool", bufs=2))
            k.ypool = ctx.enter_context(tc.tile_pool(name="ypool", bufs=2))
            k.small = ctx.enter_context(tc.tile_pool(name="small", bufs=2))
            k.psA = ctx.enter_context(tc.tile_pool(name="psA", bufs=2,
                                                   space="PSUM"))
            k.psB = ctx.enter_context(tc.tile_pool(name="psB", bufs=2,
                                                   space="PSUM"))
            k.psC = ctx.enter_context(tc.tile_pool(name="psC", bufs=4,
                                                   space="PSUM"))
            k.sb = {}

            # ---- warmup: pre-warm the HAM clock gate while sample-0's
            # first x8 piece streams in (these matmuls depend only on the
            # memset, so they start immediately). ---------------------------
            _mark(nc, "warmup")
            warm_sb = k.consts.tile([128, 512], BF16, name="warm_sb")
            nc.vector.memset(warm_sb, 0.01)
            k.sb["warm"] = warm_sb
            if NWARM:
                warm_ps = k.psA.tile([128, 512], F32, name="warm_ps", tag="pa")
                for _ in range(NWARM):
                    nc.tensor.matmul(warm_ps[:, 0:128],
                                     lhsT=warm_sb[:, 0:128],
                                     rhs=warm_sb[:, 0:128],
                                     start=True, stop=True)

            # ---- loads: wire order is the critical path. DMA queues cap at
            # ~160 GB/s each, so big tensors are split across queues and
            # issued in need-order. -----------------------------------------
            _mark(nc, "loads")
            x0 = k.xpool.tile([128, NCH, 2, 4, 512], BF16, name="x_sb_0",
                              tag="x")
            x1 = k.xpool.tile([128, NCH, 2, 4, 512], BF16, name="x_sb_1",
                              tag="x")
            x81 = k.x8pool.tile([128, NCH, NT, 128], FP8, name="x8_sb_1",
                                tag="x8")
            attT0 = k.attTpool.tile([128, NT, C8], FP8, name="attT_0",
                                    tag="attT")
            attT1 = k.attTpool.tile([128, NT, C8], FP8, name="attT_1",
                                    tag="attT")
            att20 = k.att2pool.tile([128, HW // 2], FP8, name="att2_0",
                                    tag="att2")
            att21 = k.att2pool.tile([128, HW // 2], FP8, name="att2_1",
                                    tag="att2")
            w2bT_sb = k.consts.tile([128, 4, NCH, 128], BF16, name="w2bT_sb")
            k.sb["w2bT"] = w2bT_sb

            const_specs = {
                "kwT": ([128, NCH, C8], BF16),
                "kb_bc": ([128, 8, C8], BF16),
                "wa_blob": ([128, 640], BF16),
                "bias_blob": ([128, 5], F32),
            }

            def load_consts(eng, names):
                for name in names:
                    shape, dt = const_specs[name]
                    t = k.consts.tile(shape, dt, name=f"{name}_sb")
                    eng.dma_start(out=t, in_=k.dram[name][:])
                    k.sb[name] = t

            def xq(dst, s, q, half):
                # half a pixel-quarter of the bf16 x (512 KB pieces so two
                # DMA queues stream one quarter in parallel)
                h, u = q // 2, q % 2
                nc.sync.dma_start(
                    out=dst[:, :, h, 2 * u + half, :],
                    in_=k.dram["x"][s][:, :, h, 2 * u + half, :])

            # Small head-critical loads go on the Scalar engine's HWDGE ring
            # (qActDynamicHW): ACT is idle in the head, and this keeps them
            # off the Sync ring where the big x pieces would delay them.
            load_consts(nc.scalar, ["kwT"])
            nc.scalar.dma_start(out=attT0, in_=k.dram["attT8"][0])
            load_consts(nc.scalar, ["kb_bc", "bias_blob", "wa_blob"])
            nc.scalar.dma_start(out=att20, in_=k.dram["att28"][0])

            # Sync ring, in need-order: both samples' attention inputs first
            # (both attentions run inside the DMA-bound head), then the c2
            # weights, then sample-1's bf16 x.
            xq(x0, 0, 0, 0)
            xq(x0, 0, 0, 1)
            nc.sync.dma_start(out=x81[:, :, 0:16, :],
                              in_=k.dram["x8"][:, :, 0:16, :])
            xq(x0, 0, 1, 0)
            xq(x0, 0, 1, 1)
            nc.sync.dma_start(out=x81[:, :, 16:32, :],
                              in_=k.dram["x8"][:, :, 16:32, :])
            xq(x0, 0, 2, 0)
            xq(x0, 0, 2, 1)
            nc.sync.dma_start(out=attT1, in_=k.dram["attT8"][1])
            xq(x0, 0, 3, 0)
            xq(x0, 0, 3, 1)
            nc.sync.dma_start(out=att21, in_=k.dram["att28"][1])
            nc.sync.dma_start(out=w2bT_sb[:, 0], in_=k.dram["w2bT"][:, 0])
            nc.sync.dma_start(out=w2bT_sb[:, 1:4], in_=k.dram["w2bT"][:, 1:4])
            nc.sync.dma_start(out=x1[:, :, 0, :, :],
                              in_=k.dram["x"][1][:, :, 0, :, :])
            nc.sync.dma_start(out=x1[:, :, 1, :, :],
                              in_=k.dram["x"][1][:, :, 1, :, :])

            # ---- head: BOTH samples' attention, interleaved by quarter and
            # paced by the DMA; fillers keep the clock gate open in gaps. ---
            xt0 = lambda ci, nt: _xtile_bf(x0, ci, nt)
            xt1 = lambda ci, nt: x81[:, ci, nt, :]
            ps_e0 = k.psB.tile([C8, C8], F32, name="ps_e_0", tag="sm")
            # ps_e1 lives in psC: psB must stay a clean 2-deep rotation for
            # the softmax/out2 tiles (ps_e1 in psB would deadlock the ACT
            # queue: out2_0's psum would wait on ps_e1, freed only by
            # sample-1's exp, which sits behind out2_0's relu in the queue)
            ps_e1 = k.psC.tile([C8, C8], F32, name="ps_e_1", tag="c2")
            for q in range(4):
                pj = _emit_projT_quarter(k, 0, q, xt0)
                _emit_energy_quarter(k, 0, q, pj, attT0, ps_e0)
                if q < 3:
                    pj = _emit_projT_quarter(k, 1, q, xt1)
                    _emit_energy_quarter(k, 1, q, pj, attT1, ps_e1)
                    if NFILL:
                        _emit_filler(k, NFILL, 512)

            # prefill pair (ot0, j0/j4) fills the softmax-0 latency window
            _mark(nc, "prefill")
            pf_l = k.psC.tile([128, 512], F32, name="ps_y_0_0_0", tag="c2")
            pf_h = k.psC.tile([128, 512], F32, name="ps_y_0_0_4", tag="c2")
            _emit_c2x_pair(k, pf_l, pf_h, 0, 0, x0)

            w1aT_0 = _emit_softmax(k, 0, ps_e0)

            # sample-1's last attention quarter runs under softmax-0's
            # cross-engine latency
            pj = _emit_projT_quarter(k, 1, 3, xt1)
            _emit_energy_quarter(k, 1, 3, pj, attT1, ps_e1)

            _mark(nc, "out2_0")
            out20 = k.out2pool.tile([128, HW // 2], BF16, name="out2_0",
                                    tag="out2")
            _emit_out2_jb(k, 0, w1aT_0, att20, out20, 0)

            _mark(nc, "prefill_fin")
            y00 = _get_y(k, 0, 0)
            done00 = k.jdone.setdefault((0, 0), [])
            _emit_c2o_mm(k, pf_l, 0, 0, out20, hi=False)
            _emit_c2o_mm(k, pf_h, 0, 4, out20, hi=True)
            _emit_y_evac(k, 0, pf_l, 0, 0, y00, use_act=True)
            _emit_y_evac(k, 0, pf_h, 0, 4, y00, use_act=False)
            done00 += [0, 4]
            for jb in (1, 2, 3):
                _emit_out2_jb(k, 0, w1aT_0, att20, out20, jb)

            w1aT_1 = _emit_softmax(k, 1, ps_e1)

            # ---- c2 stream: softmax-1's latency hides under c2_0_0 -------
            _emit_c2_ot(k, 0, 0, x0, out20, skip_jj=(0,))

            _mark(nc, "out2_1")
            out21 = k.out2pool.tile([128, HW // 2], BF16, name="out2_1",
                                    tag="out2")
            for jb in range(4):
                _emit_out2_jb(k, 1, w1aT_1, att21, out21, jb)

            _emit_c2_ot(k, 0, 1, x0, out20, wide_psum=True)
            _emit_c2_ot(k, 0, 2, x0, out20, wide_psum=True)
            _emit_c2_ot(k, 0, 3, x0, out20, wide_psum=True)

            for ot in range(4):
                _emit_c2_ot(k, 1, ot, x1, out21, wide_psum=True,
                            quarters=(ot == 2), pair_stores=(ot == 3),
                            last=(ot == 3))

    nc.finalize()
    return nc


def _get_built():
    global _BUILT
    if _BUILT is None:
        _BUILT = _build()
    return _BUILT


def _prep_weights(key_w, key_b, c1_w, c1_b, c1_gamma, c1_beta, c1_mean, c1_var,
                  c2_w, c2_b, c2_gamma, c2_beta, c2_mean, c2_var):
    s1 = c1_gamma / np.sqrt(c1_var + EPS)
    w1 = c1_w * s1[:, None]                       # (64, 64)
    b1 = c1_b * s1 + c1_beta - c1_mean * s1       # (64,)
    s2 = c2_gamma / np.sqrt(c2_var + EPS)
    w2 = c2_w * s2[:, None]                       # (512, 576)
    b2 = c2_b * s2 + c2_beta - c2_mean * s2       # (512,)
    w2a = w2[:, :C8]                              # (512, 64)  applies to out2
    w2b = w2[:, C8:]                              # (512, 512) applies to x

    w2aT = np.ascontiguousarray(w2a.T)            # (64, 512)
    w1T = np.ascontiguousarray(w1.T)              # (64, 64)

    # wa_blob [128, 640]: cols 0:512 = w2aT duplicated into both partition
    # halves; cols 512:640 = [w1T | w1T] on partitions 0:64 (junk elsewhere).
    wa_blob = np.zeros((128, 640), np.float32)
    wa_blob[0:64, 0:512] = w2aT
    wa_blob[64:128, 0:512] = w2aT
    wa_blob[0:64, 512:576] = w1T
    wa_blob[0:64, 576:640] = w1T
    # bias_blob [128, 5]: col 0 = b1 duplicated; cols 1:5 = b2 as (4,128).T
    bias_blob = np.zeros((128, 5), np.float32)
    bias_blob[:, 0] = np.concatenate([b1, b1])
    bias_blob[:, 1:5] = b2.reshape(4, 128).T
    # w2bT [p, ot, ci, oc] = w2b[ot*128+oc, ci*128+p]
    w2bT = np.ascontiguousarray(
        w2b.reshape(4, 128, NCH, 128).transpose(3, 0, 2, 1))
    return {
        "kwT": np.ascontiguousarray(
            key_w.T.reshape(NCH, 128, C8).transpose(1, 0, 2)).astype(NP_BF16),
        "kb_bc": np.ascontiguousarray(
            np.broadcast_to(key_b[None, None, :], (128, 8, C8))).astype(NP_BF16),
        "wa_blob": wa_blob.astype(NP_BF16),
        "bias_blob": bias_blob.astype(np.float32),
        "w2bT": w2bT.astype(NP_BF16),
    }


def _prep_in_maps(inputs):
    x = np.asarray(inputs["x"], np.float32).reshape(B, C, HW)
    att = np.asarray(inputs["att"], np.float32).reshape(B, C8, HW)
    weights = _prep_weights(**{kk: np.asarray(v, np.float32)
                               for kk, v in inputs.items()
                               if kk not in ("x", "att")})
    in_maps = []
    for c in range(N_CORES):
        s0 = c * S
        # x_core[s, p, ci, h, jj, u] = x[s0+s, ci*128 + p, h*2048 + jj*512 + u]
        x_perm = np.ascontiguousarray(
            x[s0:s0 + S].reshape(S, NCH, 128, HW).transpose(0, 2, 1, 3))
        x_core = x_perm.reshape(S, 128, NCH, 2, 4, 512).astype(NP_BF16)
        # x8[p, ci, nt, u] = x[s0+1, ci*128 + p, nt*128 + u]  (sample 1 only)
        x8_core = np.ascontiguousarray(
            x_perm[1]).reshape(128, NCH, NT, 128).astype(NP_FP8)
        att_c = att[s0:s0 + S]                       # (S, 64, HW)
        # attT[s, p, nt, q] = att[s, q, nt*128 + p]
        attT_core = np.ascontiguousarray(
            att_c.reshape(S, C8, NT, 128).transpose(0, 3, 2, 1)
        ).astype(NP_FP8)
        # att2[s, 0:64, n] = att[s, :, n]; att2[s, 64:128, n] = att[s, :, 2048+n]
        att2_core = np.ascontiguousarray(
            att_c.reshape(S, C8, 2, HW // 2).transpose(0, 2, 1, 3)
            .reshape(S, 128, HW // 2)).astype(NP_FP8)
        m = {"x": x_core, "x8": x8_core, "attT8": attT_core, "att28": att2_core}
        m.update(weights)
        in_maps.append(m)
    return in_maps


def kernel(**inputs):
    nc = _get_built()
    in_maps = _prep_in_maps(inputs)
    res = run_bass_kernel_spmd(nc, in_maps, core_ids=list(range(N_CORES)))
    y = np.concatenate([np.asarray(res.results[c]["y"], dtype=np.float32)
                        for c in range(N_CORES)], axis=0)
    return np.ascontiguousarray(y.reshape(B, C, H, W)).astype(np.float32)
